# revision 21
# baseline (speedup 1.0000x reference)
"""Trainium2 Bass kernel for the NodeEdge GNN message-passing module.

Computes  out[b,n,h] = sum_e (w*inci + b)[n,e] * relu(inputs @ W_xes + b_xes)[b,e,h]
with B=16, N=2048, E=8192, DIM=64, DH=32.

Strategy: shard the edge (contraction) dimension E across the 8 NeuronCores
(EC=1024 edges per core); partial outputs are summed on the host.
Datapath is bf16 end to end (gate rel_err < 2e-2; this lands ~4e-3).

Key measured facts this structure is built around:
  - ~8.7us fixed preamble; single sync-queue DMA FIFO at ~0.35 MiB/us.
  - The PE runs at its instruction roofline (~222ns per N=512 bf16
    matmul) once dense; the win is starting dense earlier and keeping
    the post-stream tail short.
  - The DVE mask-multiply chain (1x mode, u8 operand) is serial and
    saturated; its completion gates the last chunks' matmuls.

Per core:
  - inputs ship PER-E-CHUNK ([kpair, d, c, j, x] layout) so xe[k] =
    relu(inp_k @ W_xes) needs only its own 256 KiB slice — the first
    matmuls start ~6us earlier than with the all-j layout.
  - PSUM (8 banks): 2 banks ("px" pool) rotate warm-up scratch and the
    8 xe accumulators; 6 banks ("ps" pool) carry the big-matmul
    accumulators through four groups:
      S1: (h0,nb0-3)+(h1,nb0-1) accumulate e-chunks 0-3, park to SBUF;
      F1: (h2,nb0-3)+(h3,nb0-1) full chains on the parked banks;
      F2: (h3,nb2-3) full chains on the px banks (free after xe7);
      F3: (h1,nb2-3) full chains on banks freed by F1 evacuations;
      S3: S1 tiles resume e-chunks 4-7, parked partial added back in
          the DVE evacuation.
  - PE emission is hand-interleaved to match operand readiness (PE
    queue is FIFO; an early instruction waiting on late data blocks
    everything behind it).
  - mask-multiplies in halves (quarters for the last two chunks) so
    late chunks unlock their matmuls with minimum latency.
"""

from contextlib import ExitStack

import ml_dtypes
import numpy as np

import concourse.bass as bass
import concourse.mybir as mybir
import concourse.tile as tile
from concourse import bacc
from concourse.bass_utils import run_bass_kernel_spmd

B, N, E, DIM = 16, 2048, 8192, 64
DH = DIM // 2              # 32
NCORES = 8
EC = E // NCORES           # 1024 edges per core
KC = EC // 128             # 8 e-chunks of 128
BH = B * DH                # 512 (flattened (b, h) output dim)
NB = N // 512              # 4 column blocks of the big matmul
NJ = B // 2                # 8 input tiles, two batch rows packed per tile
KSPLIT = KC // 2           # S1/S3 split of the contraction

F32 = mybir.dt.float32
BF16 = mybir.dt.bfloat16
U8 = mybir.dt.uint8
BF16NP = ml_dtypes.bfloat16

# tile groups (h, nb)
S1_TILES = [(1, 0), (1, 1), (0, 0), (0, 1), (0, 2), (0, 3)]
F1_TILES = [(2, 0), (3, 0), (2, 1), (3, 1), (2, 2), (2, 3)]
F2_TILES = [(3, 2), (3, 3)]
F3_TILES = [(1, 2), (1, 3)]

_PROGRAMS: dict = {}


def _build_program(with_bxes: bool, with_b: bool):
    nc = bacc.Bacc(
        "TRN2", target_bir_lowering=False, debug=False, enable_asserts=False
    )

    inp_t = nc.dram_tensor(
        "inp_t", [KC, 128, NJ, 128], BF16, kind="ExternalInput"
    ).ap()
    wq = nc.dram_tensor("wq", [KC, 128, N], BF16, kind="ExternalInput").ap()
    iq = nc.dram_tensor(
        "iq", [KC, 128, N], U8, kind="ExternalInput"
    ).ap()
    wx = nc.dram_tensor("wx", [128, 2 * DH], BF16, kind="ExternalInput").ap()
    bxr = (
        nc.dram_tensor("bxr", [128, BH], F32, kind="ExternalInput").ap()
        if with_bxes
        else None
    )
    bq = (
        nc.dram_tensor("bq", [KC, 128, N], BF16, kind="ExternalInput").ap()
        if with_b
        else None
    )
    outp = nc.dram_tensor("outp", [BH, N], BF16, kind="ExternalOutput").ap()

    with tile.TileContext(nc) as tc, ExitStack() as ctx:
        inp_pool = ctx.enter_context(tc.tile_pool(name="inp", bufs=KC // 2))
        wx_pool = ctx.enter_context(tc.tile_pool(name="wx", bufs=1))
        xe_pool = ctx.enter_context(tc.tile_pool(name="xe", bufs=KC))
        a_pool = ctx.enter_context(tc.tile_pool(name="a", bufs=KC))
        i_pool = ctx.enter_context(tc.tile_pool(name="i", bufs=KC // 2))
        park_pool = ctx.enter_context(tc.tile_pool(name="pk", bufs=len(S1_TILES)))
        out_pool = ctx.enter_context(tc.tile_pool(name="o", bufs=4))
        ps_pool = ctx.enter_context(tc.tile_pool(name="ps", bufs=6, space="PSUM"))
        px_pool = ctx.enter_context(tc.tile_pool(name="px", bufs=2, space="PSUM"))

        # ---- HAM warmup, DMA-free (px bank 0; xe1 reuses it later).
        warm_src = wx_pool.tile([128, 64], BF16, tag="warm")
        nc.gpsimd.memset(warm_src[:], 0.0)
        ps_warm = px_pool.tile([128, BH], F32, tag="px", name="ps_warm")
        for i in range(24):
            nc.tensor.matmul(
                ps_warm[0:64, 0:64],
                warm_src[:, 0:64],
                warm_src[:, 0:64],
                start=True,
                stop=True,
            )

        wx_tile = wx_pool.tile([128, 2 * DH], BF16)
        nc.sync.dma_start(wx_tile[:], wx[:])

        bx_tile = None
        if with_bxes:
            bx_tile = wx_pool.tile([128, BH], F32, tag="bx")
            nc.sync.dma_start(bx_tile[:], bxr[:])

        # ---- tiles
        inp_tiles = [
            inp_pool.tile([128, 2, NJ, 128], BF16, tag="inp", name=f"inp_{p}",
                          bufs=KC // 2)
            for p in range(KC // 2)
        ]
        a_tiles = [
            a_pool.tile([128, N], BF16, tag="a", name=f"a_{k}", bufs=KC)
            for k in range(KC)
        ]
        it_tiles = [
            i_pool.tile([128, 2, N], U8, tag="it", name=f"it_{p}", bufs=KC // 2)
            for p in range(KC // 2)
        ]

        # ---- loads: ONE priority-ordered FIFO on the sync queue.
        # Chunk-k first-need = inp pair + inci pair + wq chunk; the last
        # chunks' wq go before the last inp pair (their matmuls are
        # multiply-gated, xe6/7 only feed the very tail).
        # per-chunk triplets: inp_k (0.25 MiB), inci_k (0.25), w_k (0.5);
        # xe6/7's inputs go just before the last two weight chunks.
        nc.sync.dma_start(inp_tiles[0][:, 0], inp_t[0])
        nc.sync.dma_start(a_tiles[0][:], wq[0])
        nc.sync.dma_start(it_tiles[0][:, 0], iq[0])
        for k in range(1, 6):
            nc.sync.dma_start(inp_tiles[k // 2][:, k % 2], inp_t[k])
            nc.sync.dma_start(it_tiles[k // 2][:, k % 2], iq[k])
            nc.sync.dma_start(a_tiles[k][:], wq[k])
        nc.sync.dma_start(inp_tiles[3][:, 0], inp_t[6])
        nc.sync.dma_start(inp_tiles[3][:, 1], inp_t[7])
        for k in range(6, KC):
            nc.sync.dma_start(it_tiles[k // 2][:, k % 2], iq[k])
            nc.sync.dma_start(a_tiles[k][:], wq[k])

        # ---- helpers ------------------------------------------------
        xe_tiles = [None] * KC

        def emit_xe(k):
            # 8 matmuls into one px bank, ScalarE relu evacuates bf16.
            ps = px_pool.tile([128, BH], F32, tag="px", name=f"ps_xe_{k}")
            src = inp_tiles[k // 2]
            for j in range(NJ):
                nc.tensor.matmul(
                    ps[:, j * 2 * DH : (j + 1) * 2 * DH],
                    src[:, k % 2, j, :],
                    wx_tile[:],
                    start=True,
                    stop=True,
                )
            xt = xe_pool.tile([128, BH], BF16, tag="xt", name=f"xe_{k}", bufs=KC)
            if with_bxes:
                nc.vector.tensor_tensor(
                    xt[:], ps[:], bx_tile[:], op=mybir.AluOpType.add
                )
                nc.scalar.activation(
                    xt[:], xt[:], mybir.ActivationFunctionType.Relu
                )
            else:
                nc.scalar.activation(
                    xt[:], ps[:], mybir.ActivationFunctionType.Relu
                )
            xe_tiles[k] = xt

        def emit_mult(k, pieces):
            ipair = it_tiles[k // 2]
            step = N // pieces
            for q in range(pieces):
                sl = slice(q * step, (q + 1) * step)
                nc.vector.tensor_tensor(
                    a_tiles[k][:, sl], a_tiles[k][:, sl],
                    ipair[:, k % 2, sl],
                    op=mybir.AluOpType.mult,
                )
            if with_b:
                bt = i_pool.tile([128, N], BF16, tag="bt", bufs=2)
                nc.sync.dma_start(bt[:], bq[k])
                nc.vector.tensor_tensor(
                    a_tiles[k][:], a_tiles[k][:], bt[:],
                    op=mybir.AluOpType.add,
                )

        def emit_group_k(psmap, tiles, k, kfirst, klast):
            for (h, nb) in tiles:
                nc.tensor.matmul(
                    psmap[(h, nb)][:],
                    xe_tiles[k][:, h * 128 : (h + 1) * 128],
                    a_tiles[k][:, nb * 512 : (nb + 1) * 512],
                    start=(k == kfirst),
                    stop=(k == klast),
                )

        # ---- accumulator tiles
        ps1 = {
            t: ps_pool.tile([128, 512], F32, tag="ps", name=f"ps1_{t[0]}_{t[1]}")
            for t in S1_TILES
        }

        # ---- PE-order interleave (matches operand readiness) --------
        emit_xe(0)
        emit_xe(1)
        emit_mult(0, 4)
        emit_mult(1, 4)
        emit_group_k(ps1, S1_TILES, 0, 0, KSPLIT - 1)
        emit_xe(2)
        emit_xe(3)
        emit_mult(2, 4)
        emit_mult(3, 4)
        emit_group_k(ps1, S1_TILES, 1, 0, KSPLIT - 1)
        emit_group_k(ps1, S1_TILES, 2, 0, KSPLIT - 1)
        emit_xe(4)
        emit_xe(5)
        emit_mult(4, 4)
        emit_mult(5, 4)
        emit_group_k(ps1, S1_TILES, 3, 0, KSPLIT - 1)

        # park S1 partials (ScalarE), in S1 tile order so F1's banks
        # free in the order F1's first matmuls need them.
        park = {}
        for t in S1_TILES:
            pk = park_pool.tile([128, 512], F32, tag="pk",
                                name=f"pk_{t[0]}_{t[1]}")
            nc.scalar.activation(
                pk[:], ps1[t][:], mybir.ActivationFunctionType.Identity
            )
            park[t] = pk

        # F1: full chains on the parked banks.
        psf1 = {
            t: ps_pool.tile([128, 512], F32, tag="ps", name=f"f1_{t[0]}_{t[1]}")
            for t in F1_TILES
        }
        for k in range(4):
            emit_group_k(psf1, F1_TILES, k, 0, KC - 1)
        emit_mult(6, 4)
        emit_group_k(psf1, F1_TILES, 4, 0, KC - 1)
        emit_group_k(psf1, F1_TILES, 5, 0, KC - 1)
        emit_xe(6)
        emit_xe(7)
        emit_mult(7, 4)

        # F2: full chains on the px banks (free once xe6/7 evacuated).
        psf2 = {
            t: px_pool.tile([128, 512], F32, tag="px", name=f"f2_{t[0]}_{t[1]}")
            for t in F2_TILES
        }
        emit_group_k(psf1, F1_TILES, 6, 0, KC - 1)
        for k in range(4):
            emit_group_k(psf2, F2_TILES, k, 0, KC - 1)
        emit_group_k(psf1, F1_TILES, 7, 0, KC - 1)
        for k in range(4, 7):
            emit_group_k(psf2, F2_TILES, k, 0, KC - 1)
        emit_group_k(psf2, F2_TILES, 7, 0, KC - 1)

        # evacuate F1 -> output rows h2 (all) and h3 (nb0-1)
        ot_h = {
            h: out_pool.tile([128, N], BF16, tag="o", name=f"ot_{h}")
            for h in range(4)
        }
        for i, (h, nb) in enumerate(F1_TILES):
            dst = ot_h[h][:, nb * 512 : (nb + 1) * 512]
            if i % 2 == 0:
                nc.scalar.activation(
                    dst, psf1[(h, nb)][:],
                    mybir.ActivationFunctionType.Identity,
                )
            else:
                nc.vector.tensor_copy(dst, psf1[(h, nb)][:])
        nc.scalar.dma_start(outp[2 * 128 : 3 * 128, :], ot_h[2][:])

        # F3: full chains for (h1, nb2-3) on banks freed by F1 evacs.
        psf3 = {
            t: ps_pool.tile([128, 512], F32, tag="ps", name=f"f3_{t[0]}_{t[1]}")
            for t in F3_TILES
        }
        for k in range(KC):
            emit_group_k(psf3, F3_TILES, k, 0, KC - 1)

        # F2 evac completes h3; store it.
        for i, (h, nb) in enumerate(F2_TILES):
            dst = ot_h[h][:, nb * 512 : (nb + 1) * 512]
            if i % 2 == 0:
                nc.scalar.activation(
                    dst, psf2[(h, nb)][:],
                    mybir.ActivationFunctionType.Identity,
                )
            else:
                nc.vector.tensor_copy(dst, psf2[(h, nb)][:])
        nc.scalar.dma_start(outp[3 * 128 : 4 * 128, :], ot_h[3][:])

        # S3: S1 tiles resume e-chunks 4-7 (tile-major, h1 tiles first
        # so ot_h1 can store early), parked partial added back in the
        # DVE evacuation; h0 stores in column halves.
        pss3 = {}

        def emit_s3(t):
            pss3[t] = ps_pool.tile(
                [128, 512], F32, tag="ps", name=f"s3_{t[0]}_{t[1]}"
            )
            h, nb = t
            for k in range(KSPLIT, KC):
                nc.tensor.matmul(
                    pss3[t][:],
                    xe_tiles[k][:, h * 128 : (h + 1) * 128],
                    a_tiles[k][:, nb * 512 : (nb + 1) * 512],
                    start=(k == KSPLIT),
                    stop=(k == KC - 1),
                )
            nc.vector.tensor_tensor(
                ot_h[h][:, nb * 512 : (nb + 1) * 512],
                pss3[t][:],
                park[t][:],
                op=mybir.AluOpType.add,
            )

        emit_s3((1, 0))
        emit_s3((1, 1))
        # F3 evac completes h1; store it.
        for i, (h, nb) in enumerate(F3_TILES):
            dst = ot_h[h][:, nb * 512 : (nb + 1) * 512]
            if i % 2 == 0:
                nc.scalar.activation(
                    dst, psf3[(h, nb)][:],
                    mybir.ActivationFunctionType.Identity,
                )
            else:
                nc.vector.tensor_copy(dst, psf3[(h, nb)][:])
        nc.sync.dma_start(outp[1 * 128 : 2 * 128, :], ot_h[1][:])
        emit_s3((0, 0))
        emit_s3((0, 1))
        nc.sync.dma_start(outp[0 * 128 : 1 * 128, 0:1024], ot_h[0][:, 0:1024])
        emit_s3((0, 2))
        emit_s3((0, 3))
        nc.sync.dma_start(
            outp[0 * 128 : 1 * 128, 1024:2048], ot_h[0][:, 1024:2048]
        )

    nc.compile()
    return nc


def _get_program(with_bxes: bool, with_b: bool):
    key = (with_bxes, with_b)
    if key not in _PROGRAMS:
        _PROGRAMS[key] = _build_program(with_bxes, with_b)
    return _PROGRAMS[key]


def _prepare_in_maps(inputs, W_xes, b_xes, inci, w, b, with_bxes, with_b):
    inputs = np.asarray(inputs, dtype=np.float32)
    W_xes = np.asarray(W_xes, dtype=np.float32)
    b_xes = np.asarray(b_xes, dtype=np.float32)
    w = np.asarray(w, dtype=np.float32)
    b = np.asarray(b, dtype=np.float32)
    inci_u8 = np.asarray(inci).astype(np.uint8)

    wx_dup = np.zeros((128, 2 * DH), dtype=np.float32)
    wx_dup[0:DIM, 0:DH] = W_xes
    wx_dup[DIM : 2 * DIM, DH : 2 * DH] = W_xes
    wx_dup = wx_dup.astype(BF16NP)
    bxr = np.ascontiguousarray(
        np.broadcast_to(np.tile(b_xes, B)[None, :], (128, BH))
    ) if with_bxes else None

    in_maps = []
    for c in range(NCORES):
        sl = slice(c * EC, (c + 1) * EC)
        # [B, EC, D] -> [j, d2b(128), k, x] -> [kpair, d, c, j, x]
        t = np.ascontiguousarray(
            inputs[:, sl, :].transpose(0, 2, 1)
        ).reshape(NJ, 128, KC, 128).astype(BF16NP)
        t = np.ascontiguousarray(t.transpose(2, 1, 0, 3))
        wq_ = np.ascontiguousarray(w[:, sl].T).reshape(KC, 128, N).astype(BF16NP)
        iq_ = np.ascontiguousarray(inci_u8[:, sl].T).reshape(KC, 128, N)
        m = {"inp_t": t, "wq": wq_, "iq": iq_, "wx": wx_dup}
        if with_bxes:
            m["bxr"] = bxr
        if with_b:
            m["bq"] = np.ascontiguousarray(b[:, sl].T).reshape(
                KC, 128, N
            ).astype(BF16NP)
        in_maps.append(m)
    return in_maps


def _run(inputs, W_xes, b_xes, inci, w, b, **run_kwargs):
    with_bxes = bool(np.any(np.asarray(b_xes)))
    with_b = bool(np.any(np.asarray(b)))
    nc = _get_program(with_bxes, with_b)
    in_maps = _prepare_in_maps(inputs, W_xes, b_xes, inci, w, b, with_bxes, with_b)
    res = run_bass_kernel_spmd(
        nc, in_maps, core_ids=list(range(NCORES)), **run_kwargs
    )
    parts = np.stack(
        [r["outp"].astype(np.float32) for r in res.results]
    )  # [8, BH, N]
    out = parts.sum(axis=0)  # [BH, N]
    out = out.reshape(B, DH, N).transpose(0, 2, 1)  # [B, N, DH]
    return np.ascontiguousarray(out.astype(np.float32)), res


def kernel(inputs, W_xes, b_xes, inci, w, b):
    out, _ = _run(inputs, W_xes, b_xes, inci, w, b)
    return out


# revision 23
# speedup vs baseline: 1.0689x; 1.0689x over previous
"""Trainium2 Bass kernel for the NodeEdge GNN message-passing module.

Computes  out[b,n,h] = sum_e (w*inci + b)[n,e] * relu(inputs @ W_xes + b_xes)[b,e,h]
with B=16, N=2048, E=8192, DIM=64, DH=32.

Strategy: shard the edge (contraction) dimension E across the 8 NeuronCores
(EC=1024 edges per core); partial outputs are summed on the host.
Datapath is bf16 end to end (gate rel_err < 2e-2; this lands ~4e-3).

Key measured facts this structure is built around:
  - ~8.7us fixed preamble; single sync-queue DMA FIFO at ~0.35 MiB/us.
  - The PE runs at its instruction roofline (~222ns per N=512 bf16
    matmul) once dense; the win is starting dense earlier and keeping
    the post-stream tail short.
  - The DVE mask-multiply chain (1x mode, u8 operand) is serial and
    saturated; its completion gates the last chunks' matmuls.

Per core:
  - inputs ship PER-E-CHUNK ([kpair, d, c, j, x] layout) so xe[k] =
    relu(inp_k @ W_xes) needs only its own 256 KiB slice — the first
    matmuls start ~6us earlier than with the all-j layout.
  - PSUM (8 banks): 2 banks ("px" pool) rotate warm-up scratch and the
    8 xe accumulators; 6 banks ("ps" pool) carry the big-matmul
    accumulators through four groups:
      S1: (h0,nb0-3)+(h1,nb0-1) accumulate e-chunks 0-3, park to SBUF;
      F1: (h2,nb0-3)+(h3,nb0-1) full chains on the parked banks;
      F2: (h3,nb2-3) full chains on the px banks (free after xe7);
      F3: (h1,nb2-3) full chains on banks freed by F1 evacuations;
      S3: S1 tiles resume e-chunks 4-7, parked partial added back in
          the DVE evacuation.
  - PE emission is hand-interleaved to match operand readiness (PE
    queue is FIFO; an early instruction waiting on late data blocks
    everything behind it).
  - mask-multiplies in halves (quarters for the last two chunks) so
    late chunks unlock their matmuls with minimum latency.
"""

from contextlib import ExitStack

import ml_dtypes
import numpy as np

import concourse.bass as bass
import concourse.mybir as mybir
import concourse.tile as tile
from concourse import bacc
from concourse.bass_utils import run_bass_kernel_spmd

B, N, E, DIM = 16, 2048, 8192, 64
DH = DIM // 2              # 32
NCORES = 8
EC = E // NCORES           # 1024 edges per core
KC = EC // 128             # 8 e-chunks of 128
BH = B * DH                # 512 (flattened (b, h) output dim)
NB = N // 512              # 4 column blocks of the big matmul
NJ = B // 2                # 8 input tiles, two batch rows packed per tile
KSPLIT = KC // 2           # S1/S3 split of the contraction

F32 = mybir.dt.float32
BF16 = mybir.dt.bfloat16
U8 = mybir.dt.uint8
BF16NP = ml_dtypes.bfloat16

# tile groups (h, nb)
S1_TILES = [(0, 0), (1, 0), (0, 1), (1, 1), (0, 2), (0, 3)]
F1_TILES = [(2, 0), (3, 0), (2, 1), (3, 1), (2, 2), (2, 3)]
F2_TILES = [(3, 2), (3, 3)]
F3_TILES = [(1, 2), (1, 3)]

_PROGRAMS: dict = {}


def _build_program(with_bxes: bool, with_b: bool):
    nc = bacc.Bacc(
        "TRN2", target_bir_lowering=False, debug=False, enable_asserts=False
    )

    inp_t = nc.dram_tensor(
        "inp_t", [KC, 128, NJ, 128], BF16, kind="ExternalInput"
    ).ap()
    wq = nc.dram_tensor("wq", [KC, 128, N], BF16, kind="ExternalInput").ap()
    iq = nc.dram_tensor(
        "iq", [KC, 128, N], U8, kind="ExternalInput"
    ).ap()
    wx = nc.dram_tensor("wx", [128, 2 * DH], BF16, kind="ExternalInput").ap()
    bxr = (
        nc.dram_tensor("bxr", [128, BH], F32, kind="ExternalInput").ap()
        if with_bxes
        else None
    )
    bq = (
        nc.dram_tensor("bq", [KC, 128, N], BF16, kind="ExternalInput").ap()
        if with_b
        else None
    )
    outp = nc.dram_tensor("outp", [BH, N], BF16, kind="ExternalOutput").ap()

    with tile.TileContext(nc) as tc, ExitStack() as ctx:
        inp_pool = ctx.enter_context(tc.tile_pool(name="inp", bufs=KC // 2))
        wx_pool = ctx.enter_context(tc.tile_pool(name="wx", bufs=1))
        xe_pool = ctx.enter_context(tc.tile_pool(name="xe", bufs=KC))
        a_pool = ctx.enter_context(tc.tile_pool(name="a", bufs=KC))
        i_pool = ctx.enter_context(tc.tile_pool(name="i", bufs=KC // 2))
        park_pool = ctx.enter_context(tc.tile_pool(name="pk", bufs=len(S1_TILES)))
        out_pool = ctx.enter_context(tc.tile_pool(name="o", bufs=4))
        ps_pool = ctx.enter_context(tc.tile_pool(name="ps", bufs=6, space="PSUM"))
        px_pool = ctx.enter_context(tc.tile_pool(name="px", bufs=2, space="PSUM"))

        # ---- HAM warmup, DMA-free (px bank 0; xe1 reuses it later).
        warm_src = wx_pool.tile([128, 64], BF16, tag="warm")
        nc.gpsimd.memset(warm_src[:], 0.0)
        ps_warm = px_pool.tile([128, BH], F32, tag="px", name="ps_warm")
        for i in range(24):
            nc.tensor.matmul(
                ps_warm[0:64, 0:64],
                warm_src[:, 0:64],
                warm_src[:, 0:64],
                start=True,
                stop=True,
            )

        wx_tile = wx_pool.tile([128, 2 * DH], BF16)
        nc.sync.dma_start(wx_tile[:], wx[:])

        bx_tile = None
        if with_bxes:
            bx_tile = wx_pool.tile([128, BH], F32, tag="bx")
            nc.sync.dma_start(bx_tile[:], bxr[:])

        # ---- tiles
        inp_tiles = [
            inp_pool.tile([128, 2, NJ, 128], BF16, tag="inp", name=f"inp_{p}",
                          bufs=KC // 2)
            for p in range(KC // 2)
        ]
        a_tiles = [
            a_pool.tile([128, N], BF16, tag="a", name=f"a_{k}", bufs=KC)
            for k in range(KC)
        ]
        it_tiles = [
            i_pool.tile([128, 2, N], U8, tag="it", name=f"it_{p}", bufs=KC // 2)
            for p in range(KC // 2)
        ]

        # ---- loads: ONE priority-ordered FIFO on the sync queue.
        # Chunk-k first-need = inp pair + inci pair + wq chunk; the last
        # chunks' wq go before the last inp pair (their matmuls are
        # multiply-gated, xe6/7 only feed the very tail).
        # per-chunk triplets: inp_k (0.25 MiB), inci_k (0.25), w_k (0.5);
        # xe6/7's inputs go just before the last two weight chunks.
        for k in range(6):
            nc.sync.dma_start(inp_tiles[k // 2][:, k % 2], inp_t[k])
            nc.sync.dma_start(it_tiles[k // 2][:, k % 2], iq[k])
            nc.sync.dma_start(a_tiles[k][:], wq[k])
        nc.sync.dma_start(inp_tiles[3][:, 0], inp_t[6])
        nc.sync.dma_start(inp_tiles[3][:, 1], inp_t[7])
        for k in range(6, KC):
            nc.sync.dma_start(it_tiles[k // 2][:, k % 2], iq[k])
            nc.sync.dma_start(a_tiles[k][:, 0:1024], wq[k][:, 0:1024])
            nc.sync.dma_start(a_tiles[k][:, 1024:2048], wq[k][:, 1024:2048])

        # ---- helpers ------------------------------------------------
        xe_tiles = [None] * KC

        def emit_xe(k):
            # 8 matmuls into one px bank, ScalarE relu evacuates bf16.
            ps = px_pool.tile([128, BH], F32, tag="px", name=f"ps_xe_{k}")
            src = inp_tiles[k // 2]
            for j in range(NJ):
                nc.tensor.matmul(
                    ps[:, j * 2 * DH : (j + 1) * 2 * DH],
                    src[:, k % 2, j, :],
                    wx_tile[:],
                    start=True,
                    stop=True,
                )
            xt = xe_pool.tile([128, BH], BF16, tag="xt", name=f"xe_{k}", bufs=KC)
            if with_bxes:
                nc.vector.tensor_tensor(
                    xt[:], ps[:], bx_tile[:], op=mybir.AluOpType.add
                )
                nc.scalar.activation(
                    xt[:], xt[:], mybir.ActivationFunctionType.Relu
                )
            else:
                nc.scalar.activation(
                    xt[:], ps[:], mybir.ActivationFunctionType.Relu
                )
            xe_tiles[k] = xt

        def emit_mult(k, pieces):
            ipair = it_tiles[k // 2]
            step = N // pieces
            for q in range(pieces):
                sl = slice(q * step, (q + 1) * step)
                nc.vector.tensor_tensor(
                    a_tiles[k][:, sl], a_tiles[k][:, sl],
                    ipair[:, k % 2, sl],
                    op=mybir.AluOpType.mult,
                )
            if with_b:
                bt = i_pool.tile([128, N], BF16, tag="bt", bufs=2)
                nc.sync.dma_start(bt[:], bq[k])
                nc.vector.tensor_tensor(
                    a_tiles[k][:], a_tiles[k][:], bt[:],
                    op=mybir.AluOpType.add,
                )

        def emit_group_k(psmap, tiles, k, kfirst, klast):
            for (h, nb) in tiles:
                nc.tensor.matmul(
                    psmap[(h, nb)][:],
                    xe_tiles[k][:, h * 128 : (h + 1) * 128],
                    a_tiles[k][:, nb * 512 : (nb + 1) * 512],
                    start=(k == kfirst),
                    stop=(k == klast),
                )

        # ---- accumulator tiles
        ps1 = {
            t: ps_pool.tile([128, 512], F32, tag="ps", name=f"ps1_{t[0]}_{t[1]}")
            for t in S1_TILES
        }

        # ---- PE-order interleave (matches operand readiness) --------
        emit_xe(0)
        emit_xe(1)
        emit_mult(0, 4)
        emit_mult(1, 4)
        emit_group_k(ps1, S1_TILES, 0, 0, KSPLIT - 1)
        emit_xe(2)
        emit_xe(3)
        emit_mult(2, 4)
        emit_mult(3, 4)
        emit_group_k(ps1, S1_TILES, 1, 0, KSPLIT - 1)
        emit_group_k(ps1, S1_TILES, 2, 0, KSPLIT - 1)
        emit_xe(4)
        emit_xe(5)
        emit_mult(4, 4)
        emit_mult(5, 4)
        emit_group_k(ps1, S1_TILES, 3, 0, KSPLIT - 1)

        # park S1 partials (ScalarE), in S1 tile order so F1's banks
        # free in the order F1's first matmuls need them.
        park = {}
        for t in S1_TILES:
            pk = park_pool.tile([128, 512], F32, tag="pk",
                                name=f"pk_{t[0]}_{t[1]}")
            nc.scalar.activation(
                pk[:], ps1[t][:], mybir.ActivationFunctionType.Identity
            )
            park[t] = pk

        # F1: full chains on the parked banks.
        psf1 = {
            t: ps_pool.tile([128, 512], F32, tag="ps", name=f"f1_{t[0]}_{t[1]}")
            for t in F1_TILES
        }
        for k in range(4):
            emit_group_k(psf1, F1_TILES, k, 0, KC - 1)
        emit_mult(6, 4)
        emit_group_k(psf1, F1_TILES, 4, 0, KC - 1)
        emit_group_k(psf1, F1_TILES, 5, 0, KC - 1)
        emit_xe(6)
        emit_xe(7)
        emit_mult(7, 4)

        # F2: full chains on the px banks (free once xe6/7 evacuated).
        psf2 = {
            t: px_pool.tile([128, 512], F32, tag="px", name=f"f2_{t[0]}_{t[1]}")
            for t in F2_TILES
        }
        emit_group_k(psf1, F1_TILES, 6, 0, KC - 1)
        for k in range(4):
            emit_group_k(psf2, F2_TILES, k, 0, KC - 1)
        emit_group_k(psf1, F1_TILES, 7, 0, KC - 1)
        for k in range(4, 7):
            emit_group_k(psf2, F2_TILES, k, 0, KC - 1)
        emit_group_k(psf2, F2_TILES, 7, 0, KC - 1)

        # evacuate F1 -> output rows h2 (all) and h3 (nb0-1)
        ot_h = {
            h: out_pool.tile([128, N], BF16, tag="o", name=f"ot_{h}")
            for h in range(4)
        }
        for i, (h, nb) in enumerate(F1_TILES):
            dst = ot_h[h][:, nb * 512 : (nb + 1) * 512]
            if i % 2 == 0:
                nc.scalar.activation(
                    dst, psf1[(h, nb)][:],
                    mybir.ActivationFunctionType.Identity,
                )
            else:
                nc.vector.tensor_copy(dst, psf1[(h, nb)][:])
        nc.scalar.dma_start(outp[2 * 128 : 3 * 128, :], ot_h[2][:])

        # F3: full chains for (h1, nb2-3) on banks freed by F1 evacs.
        psf3 = {
            t: ps_pool.tile([128, 512], F32, tag="ps", name=f"f3_{t[0]}_{t[1]}")
            for t in F3_TILES
        }
        for k in range(KC):
            emit_group_k(psf3, F3_TILES, k, 0, KC - 1)

        # F2 evac completes h3; store it.
        for i, (h, nb) in enumerate(F2_TILES):
            dst = ot_h[h][:, nb * 512 : (nb + 1) * 512]
            if i % 2 == 0:
                nc.scalar.activation(
                    dst, psf2[(h, nb)][:],
                    mybir.ActivationFunctionType.Identity,
                )
            else:
                nc.vector.tensor_copy(dst, psf2[(h, nb)][:])
        nc.scalar.dma_start(outp[3 * 128 : 4 * 128, :], ot_h[3][:])

        # S3: S1 tiles resume e-chunks 4-7 (tile-major so each chain
        # starts as soon as its bank frees), parked partial added back
        # in the DVE evacuation.
        pss3 = {}
        for t in S1_TILES:
            pss3[t] = ps_pool.tile(
                [128, 512], F32, tag="ps", name=f"s3_{t[0]}_{t[1]}"
            )
            h, nb = t
            for k in range(KSPLIT, KC):
                nc.tensor.matmul(
                    pss3[t][:],
                    xe_tiles[k][:, h * 128 : (h + 1) * 128],
                    a_tiles[k][:, nb * 512 : (nb + 1) * 512],
                    start=(k == KSPLIT),
                    stop=(k == KC - 1),
                )
            nc.vector.tensor_tensor(
                ot_h[h][:, nb * 512 : (nb + 1) * 512],
                pss3[t][:],
                park[t][:],
                op=mybir.AluOpType.add,
            )

        # F3 evac completes h1.
        for i, (h, nb) in enumerate(F3_TILES):
            dst = ot_h[h][:, nb * 512 : (nb + 1) * 512]
            if i % 2 == 0:
                nc.scalar.activation(
                    dst, psf3[(h, nb)][:],
                    mybir.ActivationFunctionType.Identity,
                )
            else:
                nc.vector.tensor_copy(dst, psf3[(h, nb)][:])
        # h1 first (ready earlier; sync FIFO would otherwise head-of-line
        # block it behind h0's last add), h0 in column halves.
        nc.sync.dma_start(outp[1 * 128 : 2 * 128, :], ot_h[1][:])
        nc.sync.dma_start(outp[0 * 128 : 1 * 128, 0:1024], ot_h[0][:, 0:1024])
        nc.sync.dma_start(
            outp[0 * 128 : 1 * 128, 1024:2048], ot_h[0][:, 1024:2048]
        )

    nc.compile()
    return nc


def _get_program(with_bxes: bool, with_b: bool):
    key = (with_bxes, with_b)
    if key not in _PROGRAMS:
        _PROGRAMS[key] = _build_program(with_bxes, with_b)
    return _PROGRAMS[key]


def _prepare_in_maps(inputs, W_xes, b_xes, inci, w, b, with_bxes, with_b):
    inputs = np.asarray(inputs, dtype=np.float32)
    W_xes = np.asarray(W_xes, dtype=np.float32)
    b_xes = np.asarray(b_xes, dtype=np.float32)
    w = np.asarray(w, dtype=np.float32)
    b = np.asarray(b, dtype=np.float32)
    inci_u8 = np.asarray(inci).astype(np.uint8)

    wx_dup = np.zeros((128, 2 * DH), dtype=np.float32)
    wx_dup[0:DIM, 0:DH] = W_xes
    wx_dup[DIM : 2 * DIM, DH : 2 * DH] = W_xes
    wx_dup = wx_dup.astype(BF16NP)
    bxr = np.ascontiguousarray(
        np.broadcast_to(np.tile(b_xes, B)[None, :], (128, BH))
    ) if with_bxes else None

    in_maps = []
    for c in range(NCORES):
        sl = slice(c * EC, (c + 1) * EC)
        # [B, EC, D] -> [j, d2b(128), k, x] -> [kpair, d, c, j, x]
        t = np.ascontiguousarray(
            inputs[:, sl, :].transpose(0, 2, 1)
        ).reshape(NJ, 128, KC, 128).astype(BF16NP)
        t = np.ascontiguousarray(t.transpose(2, 1, 0, 3))
        wq_ = np.ascontiguousarray(w[:, sl].T).reshape(KC, 128, N).astype(BF16NP)
        iq_ = np.ascontiguousarray(inci_u8[:, sl].T).reshape(KC, 128, N)
        m = {"inp_t": t, "wq": wq_, "iq": iq_, "wx": wx_dup}
        if with_bxes:
            m["bxr"] = bxr
        if with_b:
            m["bq"] = np.ascontiguousarray(b[:, sl].T).reshape(
                KC, 128, N
            ).astype(BF16NP)
        in_maps.append(m)
    return in_maps


def _run(inputs, W_xes, b_xes, inci, w, b, **run_kwargs):
    with_bxes = bool(np.any(np.asarray(b_xes)))
    with_b = bool(np.any(np.asarray(b)))
    nc = _get_program(with_bxes, with_b)
    in_maps = _prepare_in_maps(inputs, W_xes, b_xes, inci, w, b, with_bxes, with_b)
    res = run_bass_kernel_spmd(
        nc, in_maps, core_ids=list(range(NCORES)), **run_kwargs
    )
    parts = np.stack(
        [r["outp"].astype(np.float32) for r in res.results]
    )  # [8, BH, N]
    out = parts.sum(axis=0)  # [BH, N]
    out = out.reshape(B, DH, N).transpose(0, 2, 1)  # [B, N, DH]
    return np.ascontiguousarray(out.astype(np.float32)), res


def kernel(inputs, W_xes, b_xes, inci, w, b):
    out, _ = _run(inputs, W_xes, b_xes, inci, w, b)
    return out
